# revision 20
# baseline (speedup 1.0000x reference)
"""Trainium2 Bass kernel for GQA causal attention (nn_Attention_83623013253180).

Shapes: B=2, L=2048, D=1024, H=16 heads, G=2 kv-groups, HPG=8, DQK=DV=128.

Sharding (8 cores): core c -> (b = c//4, g = (c%4)//2, hh = c%2), each core
handles one batch, one kv group, and 4 of that group's 8 query heads.
Wq/Wk/Wv are column-sharded, Wo row-sharded; the out-proj all-reduce (sum of
4 partials per batch) is done on host after gather, along with + bo.

Per-core device kernel (all matmul operands fp16, PSUM fp32), organized as a
pipeline over q chunks (small first chunks so attention starts as soon as a
sliver of x has landed):
  chunk: load x columns -> kT/qT/V_aug projections -> causal attention for
  the chunk (kv tiles 0..end) -> out projection -> store fp16.

  - x arrives chunk-major from host: [128, sum(8*qw)] fp16, contiguous per
    chunk so every load is a single wide DMA row per partition
  - qT[h] = (Wq_h^T X^T)  [128 dqk, tok]   (lhsT=Wq tile, rhs=xT)
  - S^T tile = matmul(lhsT=kT slice, rhs=qT cols) -> PSUM [128 kv, qw]
  - e = exp(S^T * scale) on ScalarE -> fp16 SBUF; causal 0/1 mask multiply
    on diagonal tiles (DVE)
  - ctx PSUM [q 128, 129] += matmul(lhsT=e slice, rhs=V_aug tile); V_aug has
    a ones column so col 128 accumulates the softmax denominator
  - normalize per-partition via reciprocal, PE-transpose ctx -> ctxT
  - out[q,1024] partial = sum_h matmul(lhsT=ctxT_h, rhs=Wo_h) -> DMA fp16

Warmup: dummy matmuls on memset data ramp the PE clock while the first DMAs
stream; chunk-0 loads are split across both HWDGE trigger queues
(sync+scalar) and halved so projections start on partial data.
"""

import numpy as np

import concourse.bass as bass
import concourse.mybir as mybir
import concourse.tile as tile
from concourse import bacc
from concourse.bass_utils import run_bass_kernel_spmd

F16 = mybir.dt.float16
F32 = mybir.dt.float32

B, L, D = 2, 2048, 1024
H, G, HPG = 16, 2, 8
DQK = DV = 128
NHEAD = 4          # heads per core
NDT = D // 128     # 8 contraction tiles over input dim
NKV = L // 128     # 16 kv tiles
NCORES = 8
NWARM = 5          # dummy matmuls to ramp the PE clock during warmup
CHUNKS = [(0, 256), (256, 256), (512, 512), (1024, 512), (1536, 512)]
# flat chunk-major column offset of each chunk in the x dram tensors
_XOFF = [0]
for _, _qw in CHUNKS:
    _XOFF.append(_XOFF[-1] + NDT * _qw)
XTOT = _XOFF[-1]   # == NDT * L


def _build(scale_val: float) -> bass.Bass:
    nc = bacc.Bacc("TRN2", target_bir_lowering=False, debug=False, num_devices=NCORES)

    xq = nc.dram_tensor("xqT", [128, XTOT], F16, kind="ExternalInput")
    xk = nc.dram_tensor("xkT", [128, XTOT], F16, kind="ExternalInput")
    xv = nc.dram_tensor("xvT", [128, XTOT], F16, kind="ExternalInput")
    wq = nc.dram_tensor("wq", [NHEAD, 128, NDT, DQK], F16, kind="ExternalInput")
    wk = nc.dram_tensor("wk", [128, NDT, DQK], F16, kind="ExternalInput")
    wv = nc.dram_tensor("wv", [128, NDT, DV + 1], F16, kind="ExternalInput")
    wo = nc.dram_tensor("wo", [128, NHEAD, D], F16, kind="ExternalInput")
    bq = nc.dram_tensor("bq", [128, NHEAD], F32, kind="ExternalInput")
    bk = nc.dram_tensor("bk", [128, 1], F32, kind="ExternalInput")
    bvb = nc.dram_tensor("bvb", [128, DV + 1], F32, kind="ExternalInput")
    msk = nc.dram_tensor("msk", [128, 128], F16, kind="ExternalInput")
    idn = nc.dram_tensor("idn", [128, 128], F16, kind="ExternalInput")
    out = nc.dram_tensor("out", [L, D], F16, kind="ExternalOutput")

    with tile.TileContext(nc) as tc:
        with (
            tc.tile_pool(name="const", bufs=1) as cpool,
            tc.tile_pool(name="xbuf", bufs=1) as xpool,
            tc.tile_pool(name="qkv", bufs=1) as qkvpool,
            tc.tile_pool(name="work", bufs=8) as wpool,
            tc.tile_pool(name="masked", bufs=4) as mpool,
            tc.tile_pool(name="ctxt", bufs=3) as ctpool,
            tc.tile_pool(name="outb", bufs=4) as opool,
            tc.tile_pool(name="ps_a", bufs=4, space="PSUM") as ps_a,
            tc.tile_pool(name="ps_ctx", bufs=4, space="PSUM") as ps_ctx,
        ):
            wk_sb = cpool.tile([128, NDT, DQK], F16, tag="wk")
            bk_sb = cpool.tile([128, 1], F32, tag="bk")
            bq_sb = cpool.tile([128, NHEAD], F32, tag="bq")
            bvb_sb = cpool.tile([128, DV + 1], F32, tag="bvb")
            msk_sb = cpool.tile([128, 128], F16, tag="msk")
            idn_sb = cpool.tile([128, 128], F16, tag="idn")
            wq_sb = cpool.tile([128, NHEAD, NDT, DQK], F16, tag="wq")
            wv_sb = cpool.tile([128, NDT, DV + 1], F16, tag="wv")
            wo_sb = cpool.tile([128, NHEAD, D], F16, tag="wo")
            warm_sb = cpool.tile([128, 128 + 512], F16, tag="warm")

            q_sb = qkvpool.tile([128, NHEAD, L], F16, tag="q")    # qT per head
            k_sb = qkvpool.tile([128, L], F16, tag="k")           # kT
            v_sb = qkvpool.tile([128, NKV, DV + 1], F16, tag="v")  # V_aug tiles

            xq_sb = xpool.tile([128, XTOT], F16, tag="xq")
            xk_sb = xpool.tile([128, XTOT], F16, tag="xk")
            xv_sb = xpool.tile([128, XTOT], F16, tag="xv")

            def load_chunk(ci):
                fo, fe = _XOFF[ci], _XOFF[ci + 1]
                if ci == 0:
                    # chunk 0 is latency-critical: triggers split across the
                    # two HWDGE queues (sync + scalar), x split in half so
                    # compute starts on partial data
                    mid = (fo + fe) // 2
                    nc.sync.dma_start(wk_sb[:], wk[:])
                    nc.sync.dma_start(xk_sb[:, fo:mid], xk[:, fo:mid])
                    nc.sync.dma_start(xk_sb[:, mid:fe], xk[:, mid:fe])
                    nc.sync.dma_start(wv_sb[:], wv[:])
                    nc.sync.dma_start(xv_sb[:, fo:mid], xv[:, fo:mid])
                    nc.sync.dma_start(xv_sb[:, mid:fe], xv[:, mid:fe])
                    nc.sync.dma_start(bk_sb[:], bk[:])
                    nc.sync.dma_start(bvb_sb[:], bvb[:])
                    nc.sync.dma_start(msk_sb[:], msk[:])
                    nc.sync.dma_start(idn_sb[:], idn[:])
                    nc.scalar.dma_start(wq_sb[:, 0], wq[0])
                    nc.scalar.dma_start(xq_sb[:, fo:mid], xq[:, fo:mid])
                    nc.scalar.dma_start(xq_sb[:, mid:fe], xq[:, mid:fe])
                    for hi in range(1, NHEAD):
                        nc.scalar.dma_start(wq_sb[:, hi], wq[hi])
                    nc.scalar.dma_start(bq_sb[:], bq[:])
                    nc.scalar.dma_start(wo_sb[:], wo[:])
                else:
                    nc.sync.dma_start(xk_sb[:, fo:fe], xk[:, fo:fe])
                    nc.sync.dma_start(xq_sb[:, fo:fe], xq[:, fo:fe])
                    nc.sync.dma_start(xv_sb[:, fo:fe], xv[:, fo:fe])

            def xsl(x_sb, ci, dt_i, lo, hi_):
                """AP for x columns [lo,hi_) of chunk ci's dt tile."""
                _, qw = CHUNKS[ci]
                base = _XOFF[ci] + dt_i * qw
                return x_sb[:, base + lo:base + hi_]

            # ---- PE clock pre-warm: dummy matmuls on memset data ----
            nc.gpsimd.memset(warm_sb[:], 0.0)
            for wi in range(NWARM):
                wp = ps_a.tile([128, 512], F32, tag="ps_a", name=f"warm{wi}")
                nc.tensor.matmul(
                    wp, warm_sb[:, 0:128], warm_sb[:, 128:128 + 512],
                    start=True, stop=True,
                )

            for ci, (q0, qw) in enumerate(CHUNKS):
                sl = slice(q0, q0 + qw)
                nj = qw // 128           # q sub-tiles in this chunk
                jt0 = q0 // 128          # first q tile index
                nkv_c = jt0 + nj         # kv tiles visible to this chunk

                load_chunk(ci)

                # ---- K projection ----
                pk = ps_a.tile([128, qw], F32, tag="ps_a", name=f"pk{ci}")
                for dt_i in range(NDT):
                    nc.tensor.matmul(
                        pk, wk_sb[:, dt_i, :], xsl(xk_sb, ci, dt_i, 0, qw),
                        start=(dt_i == 0), stop=(dt_i == NDT - 1),
                    )
                nc.vector.tensor_tensor(
                    k_sb[:, sl], pk, bk_sb[:].to_broadcast((128, qw)),
                    mybir.AluOpType.add,
                )

                # ---- Q projection (per head) ----
                for hi in range(NHEAD):
                    pq = ps_a.tile([128, qw], F32, tag="ps_a",
                                   name=f"pq{ci}_{hi}")
                    for dt_i in range(NDT):
                        nc.tensor.matmul(
                            pq, wq_sb[:, hi, dt_i, :],
                            xsl(xq_sb, ci, dt_i, 0, qw),
                            start=(dt_i == 0), stop=(dt_i == NDT - 1),
                        )
                    nc.vector.tensor_tensor(
                        q_sb[:, hi, sl], pq,
                        bq_sb[:, hi:hi + 1].to_broadcast((128, qw)),
                        mybir.AluOpType.add,
                    )

                # ---- V projection (per kv tile of this chunk) ----
                for kvs in range(nj):
                    kv = jt0 + kvs
                    pv = ps_a.tile([128, DV + 1], F32, tag="ps_a",
                                   name=f"pv{ci}_{kvs}")
                    for dt_i in range(NDT):
                        nc.tensor.matmul(
                            pv, xsl(xv_sb, ci, dt_i, kvs * 128,
                                    (kvs + 1) * 128),
                            wv_sb[:, dt_i, :],
                            start=(dt_i == 0), stop=(dt_i == NDT - 1),
                        )
                    nc.vector.tensor_tensor(
                        v_sb[:, kv, :], pv, bvb_sb[:], mybir.AluOpType.add
                    )

                # ---- attention for this chunk (kv tiles 0..nkv_c-1) ----
                ctxT = ctpool.tile([128, NHEAD, nj, 128], F16, tag="ctxT",
                                   name=f"ctxT{ci}")
                for hi in range(NHEAD):
                    ctx_ps = [
                        ps_ctx.tile([128, DV + 1], F32, tag="ctx",
                                    name=f"ctx_{ci}_{hi}_{j}")
                        for j in range(nj)
                    ]
                    for kv in range(nkv_c):
                        t_off = kv * 128 - q0   # kv tile col offset in chunk
                        # causal: q columns below kv tile start are all
                        # masked -> shrink score/exp width to the live part
                        qoff = max(t_off, 0)
                        w = qw - qoff
                        s_ps = ps_a.tile([128, qw], F32, tag="ps_a",
                                         name=f"s_{ci}_{hi}_{kv}")
                        nc.tensor.matmul(
                            s_ps[:, :w],
                            k_sb[:, kv * 128:(kv + 1) * 128],
                            q_sb[:, hi, q0 + qoff:q0 + qw],
                            start=True, stop=True,
                        )
                        e_sb = wpool.tile([128, qw], F16, tag="e",
                                          name=f"e_{ci}_{hi}_{kv}")
                        nc.scalar.activation(
                            e_sb[:, :w], s_ps[:, :w],
                            mybir.ActivationFunctionType.Exp,
                            bias=0.0, scale=scale_val,
                        )
                        if t_off >= 0:
                            # only the leading 128 block straddles the
                            # diagonal; later blocks are fully allowed
                            em_sb = mpool.tile([128, 128], F16, tag="em")
                            nc.vector.tensor_tensor(
                                em_sb[:], e_sb[:, 0:128], msk_sb[:],
                                mybir.AluOpType.mult,
                            )
                        for j in range(nj):
                            if kv > jt0 + j:
                                continue
                            if t_off == j * 128:
                                e_use = em_sb[:, 0:128]
                            else:
                                e_use = e_sb[:, j * 128 - qoff:
                                             (j + 1) * 128 - qoff]
                            nc.tensor.matmul(
                                ctx_ps[j],
                                e_use,
                                v_sb[:, kv, :],
                                start=(kv == 0), stop=(kv == jt0 + j),
                            )
                    for j in range(nj):
                        rcp = wpool.tile([128, 1], F32, tag="rcp")
                        nc.vector.reciprocal(rcp[:], ctx_ps[j][:, DV:DV + 1])
                        ctxn = wpool.tile([128, 128], F16, tag="ctxn")
                        nc.vector.tensor_tensor(
                            ctxn[:], ctx_ps[j][:, 0:DV],
                            rcp[:].to_broadcast((128, DV)),
                            mybir.AluOpType.mult,
                        )
                        tr_ps = ps_a.tile([128, 512], F16, tag="ps_a",
                                          name=f"tr_{ci}_{hi}_{j}")[:, 0:128]
                        nc.tensor.transpose(tr_ps, ctxn[:], idn_sb[:])
                        nc.vector.tensor_copy(ctxT[:, hi, j, :], tr_ps)

                # ---- out projection for this q chunk ----
                for j in range(nj):
                    o_sb = opool.tile([128, D], F16, tag="o")
                    for nch in range(2):
                        po = ps_a.tile([128, 512], F32, tag="ps_a",
                                       name=f"po{ci}_{j}_{nch}")
                        for hi in range(NHEAD):
                            nc.tensor.matmul(
                                po,
                                ctxT[:, hi, j, :],
                                wo_sb[:, hi, nch * 512:(nch + 1) * 512],
                                start=(hi == 0), stop=(hi == NHEAD - 1),
                            )
                        nc.vector.tensor_copy(
                            o_sb[:, nch * 512:(nch + 1) * 512], po
                        )
                        qt = jt0 + j
                        nc.sync.dma_start(
                            out[qt * 128:(qt + 1) * 128,
                                nch * 512:(nch + 1) * 512],
                            o_sb[:, nch * 512:(nch + 1) * 512],
                        )

    nc.finalize()
    return nc


_NC_CACHE: dict[float, bass.Bass] = {}


def _get_nc(scale_val: float) -> bass.Bass:
    if scale_val not in _NC_CACHE:
        _NC_CACHE[scale_val] = _build(scale_val)
    return _NC_CACHE[scale_val]


def _part_tile(a: np.ndarray) -> np.ndarray:
    """[K, F] -> [128, K//128, F] partition-tiled fp16 contiguous."""
    k, f = a.shape
    return np.ascontiguousarray(
        a.reshape(k // 128, 128, f).transpose(1, 0, 2).astype(np.float16)
    )


def _chunk_flat(a: np.ndarray) -> np.ndarray:
    """[D, L] -> [128, sum(NDT*qw)] chunk-major flat partition-tiled fp16."""
    pt = _part_tile(a)  # [128, NDT, L]
    parts = [
        pt[:, :, q0:q0 + qw].reshape(128, NDT * qw) for q0, qw in CHUNKS
    ]
    return np.ascontiguousarray(np.concatenate(parts, axis=1))


def run(inputs: dict, trace: bool = False):
    in_q = np.asarray(inputs["in_q"], np.float32)
    in_k = np.asarray(inputs["in_k"], np.float32)
    in_v = np.asarray(inputs["in_v"], np.float32)
    Wq = np.asarray(inputs["Wq"], np.float32)
    Wk = np.asarray(inputs["Wk"], np.float32)
    Wv = np.asarray(inputs["Wv"], np.float32)
    Wo = np.asarray(inputs["Wo"], np.float32)
    bq = np.asarray(inputs["bq"], np.float32)
    bk = np.asarray(inputs["bk"], np.float32)
    bv = np.asarray(inputs["bv"], np.float32)
    bo = np.asarray(inputs["bo"], np.float32)
    qes = float(np.asarray(inputs["q_extra_scale"], np.float32).reshape(-1)[0])

    scale_val = qes / float(np.sqrt(DQK))
    nc = _get_nc(scale_val)

    # triangular mask for the single diagonal 128x128 block
    ii = np.arange(128)[:, None]
    jj = np.arange(128)[None, :]
    masks = (jj >= ii).astype(np.float16)  # [128, 128], 1 where q >= kv
    idn = np.eye(128, dtype=np.float16)

    in_maps = []
    for c in range(NCORES):
        b, g, hh = c // 4, (c % 4) // 2, c % 2
        h0 = g * HPG + hh * NHEAD
        wv_aug = np.concatenate(
            [Wv[:, g * DV:(g + 1) * DV], np.zeros((D, 1), np.float32)], axis=1
        )
        bv_aug = np.concatenate([bv[g * DV:(g + 1) * DV], [1.0]]).astype(np.float32)
        wo_slice = Wo[h0 * DV:(h0 + NHEAD) * DV, :]  # [512, 1024]
        in_maps.append({
            "xqT": _chunk_flat(in_q[b].T),
            "xkT": _chunk_flat(in_k[b].T),
            "xvT": _chunk_flat(in_v[b].T),
            "wq": np.stack([
                _part_tile(Wq[:, (h0 + h) * DQK:(h0 + h + 1) * DQK])
                for h in range(NHEAD)
            ]),
            "wk": _part_tile(Wk[:, g * DQK:(g + 1) * DQK]),
            "wv": _part_tile(wv_aug),
            "wo": np.ascontiguousarray(
                wo_slice.reshape(NHEAD, DV, D).transpose(1, 0, 2).astype(np.float16)
            ),
            "bq": np.ascontiguousarray(
                bq[h0 * DQK:(h0 + NHEAD) * DQK].reshape(NHEAD, DQK).T.astype(np.float32)
            ),
            "bk": bk[g * DQK:(g + 1) * DQK].reshape(DQK, 1).astype(np.float32),
            "bvb": np.ascontiguousarray(
                np.broadcast_to(bv_aug, (128, DV + 1)).astype(np.float32)
            ),
            "msk": masks,
            "idn": idn,
        })

    res = run_bass_kernel_spmd(
        nc, in_maps, core_ids=list(range(NCORES)), trace=trace
    )

    out_full = np.zeros((B, L, D), np.float32)
    for c in range(NCORES):
        out_full[c // 4] += np.asarray(res.results[c]["out"], np.float32)
    out_full += bo
    return out_full, res.exec_time_ns


def kernel(**inputs) -> np.ndarray:
    out, _ = run(inputs, trace=False)
    return out


# revision 21
# speedup vs baseline: 1.2421x; 1.2421x over previous
"""Trainium2 Bass kernel for GQA causal attention (nn_Attention_83623013253180).

Shapes: B=2, L=2048, D=1024, H=16 heads, G=2 kv-groups, HPG=8, DQK=DV=128.

Sharding (8 cores): core c -> (b = c//4, g = (c%4)//2, hh = c%2), each core
handles one batch, one kv group, and 4 of that group's 8 query heads.
Wq/Wk/Wv are column-sharded, Wo row-sharded; the out-proj all-reduce (sum of
4 partials per batch) is done on host after gather, along with + bo.

Per-core device kernel (all matmul operands fp16, PSUM fp32), organized as a
pipeline over q chunks (small first chunks so attention starts as soon as a
sliver of x has landed):
  chunk: load x columns -> kT/qT/V_aug projections -> causal attention for
  the chunk (kv tiles 0..end) -> out projection -> store fp16.

  - x arrives chunk-major from host: [128, sum(8*qw)] fp16, contiguous per
    chunk so every load is a single wide DMA row per partition
  - qT[h] = (Wq_h^T X^T)  [128 dqk, tok]   (lhsT=Wq tile, rhs=xT)
  - S^T tile = matmul(lhsT=kT slice, rhs=qT cols) -> PSUM [128 kv, qw]
  - e = exp(S^T * scale) on ScalarE -> fp16 SBUF; causal 0/1 mask multiply
    on diagonal tiles (DVE)
  - ctx PSUM [q 128, 129] += matmul(lhsT=e slice, rhs=V_aug tile); V_aug has
    a ones column so col 128 accumulates the softmax denominator
  - normalize per-partition via reciprocal, PE-transpose ctx -> ctxT
  - out[q,1024] partial = sum_h matmul(lhsT=ctxT_h, rhs=Wo_h) -> DMA fp16

Warmup: dummy matmuls on memset data ramp the PE clock while the first DMAs
stream; chunk-0 loads are split across both HWDGE trigger queues
(sync+scalar) and halved so projections start on partial data.
"""

import numpy as np

import concourse.bass as bass
import concourse.mybir as mybir
import concourse.tile as tile
from concourse import bacc
from concourse.bass_utils import run_bass_kernel_spmd

F16 = mybir.dt.float16
F32 = mybir.dt.float32

B, L, D = 2, 2048, 1024
H, G, HPG = 16, 2, 8
DQK = DV = 128
NHEAD = 4          # heads per core
NDT = D // 128     # 8 contraction tiles over input dim
NKV = L // 128     # 16 kv tiles
NCORES = 8
NWARM = 5          # dummy matmuls to ramp the PE clock during warmup
CHUNKS = [(0, 256), (256, 256), (512, 512), (1024, 512), (1536, 512)]
# flat chunk-major column offset of each chunk in the x dram tensors
_XOFF = [0]
for _, _qw in CHUNKS:
    _XOFF.append(_XOFF[-1] + NDT * _qw)
XTOT = _XOFF[-1]   # == NDT * L


def _build(scale_val: float) -> bass.Bass:
    nc = bacc.Bacc("TRN2", target_bir_lowering=False, debug=False, num_devices=NCORES)

    xq = nc.dram_tensor("xqT", [128, XTOT], F16, kind="ExternalInput")
    xk = nc.dram_tensor("xkT", [128, XTOT], F16, kind="ExternalInput")
    xv = nc.dram_tensor("xvT", [128, XTOT], F16, kind="ExternalInput")
    wq = nc.dram_tensor("wq", [NHEAD, 128, NDT, DQK], F16, kind="ExternalInput")
    wk = nc.dram_tensor("wk", [128, NDT, DQK], F16, kind="ExternalInput")
    wv = nc.dram_tensor("wv", [128, NDT, DV + 1], F16, kind="ExternalInput")
    wo = nc.dram_tensor("wo", [128, NHEAD, D], F16, kind="ExternalInput")
    bq = nc.dram_tensor("bq", [128, NHEAD], F32, kind="ExternalInput")
    bk = nc.dram_tensor("bk", [128, 1], F32, kind="ExternalInput")
    bvb = nc.dram_tensor("bvb", [128, DV + 1], F32, kind="ExternalInput")
    msk = nc.dram_tensor("msk", [128, 128], F16, kind="ExternalInput")
    idn = nc.dram_tensor("idn", [128, 128], F16, kind="ExternalInput")
    out = nc.dram_tensor("out", [L, D], F16, kind="ExternalOutput")

    with tile.TileContext(nc) as tc:
        with (
            tc.tile_pool(name="const", bufs=1) as cpool,
            tc.tile_pool(name="xbuf", bufs=1) as xpool,
            tc.tile_pool(name="qkv", bufs=1) as qkvpool,
            tc.tile_pool(name="work", bufs=8) as wpool,
            tc.tile_pool(name="masked", bufs=4) as mpool,
            tc.tile_pool(name="ctxt", bufs=3) as ctpool,
            tc.tile_pool(name="outb", bufs=4) as opool,
            tc.tile_pool(name="ps_a", bufs=3, space="PSUM") as ps_a,
            tc.tile_pool(name="ps_ctx", bufs=4, space="PSUM") as ps_ctx,
            tc.tile_pool(name="ps_tr", bufs=1, space="PSUM") as ps_tr,
        ):
            wk_sb = cpool.tile([128, NDT, DQK], F16, tag="wk")
            bk_sb = cpool.tile([128, 1], F32, tag="bk")
            bq_sb = cpool.tile([128, NHEAD], F32, tag="bq")
            bvb_sb = cpool.tile([128, DV + 1], F32, tag="bvb")
            msk_sb = cpool.tile([128, 128], F16, tag="msk")
            idn_sb = cpool.tile([128, 128], F16, tag="idn")
            wq_sb = cpool.tile([128, NHEAD, NDT, DQK], F16, tag="wq")
            wv_sb = cpool.tile([128, NDT, DV + 1], F16, tag="wv")
            wo_sb = cpool.tile([128, NHEAD, D], F16, tag="wo")
            warm_sb = cpool.tile([128, 128 + 512], F16, tag="warm")

            q_sb = qkvpool.tile([128, NHEAD, L], F16, tag="q")    # qT per head
            k_sb = qkvpool.tile([128, L], F16, tag="k")           # kT
            v_sb = qkvpool.tile([128, NKV, DV + 1], F16, tag="v")  # V_aug tiles

            xq_sb = xpool.tile([128, XTOT], F16, tag="xq")
            xk_sb = xpool.tile([128, XTOT], F16, tag="xk")
            xv_sb = xpool.tile([128, XTOT], F16, tag="xv")

            def load_chunk(ci):
                fo, fe = _XOFF[ci], _XOFF[ci + 1]
                if ci == 0:
                    # chunk 0 is latency-critical: triggers split across the
                    # two HWDGE queues (sync + scalar), x split in half so
                    # compute starts on partial data
                    mid = (fo + fe) // 2
                    nc.sync.dma_start(wk_sb[:], wk[:])
                    nc.sync.dma_start(xk_sb[:, fo:mid], xk[:, fo:mid])
                    nc.sync.dma_start(xk_sb[:, mid:fe], xk[:, mid:fe])
                    nc.sync.dma_start(wv_sb[:], wv[:])
                    nc.sync.dma_start(xv_sb[:, fo:mid], xv[:, fo:mid])
                    nc.sync.dma_start(xv_sb[:, mid:fe], xv[:, mid:fe])
                    nc.sync.dma_start(bk_sb[:], bk[:])
                    nc.sync.dma_start(bvb_sb[:], bvb[:])
                    nc.sync.dma_start(msk_sb[:], msk[:])
                    nc.sync.dma_start(idn_sb[:], idn[:])
                    nc.scalar.dma_start(wq_sb[:, 0], wq[0])
                    nc.scalar.dma_start(xq_sb[:, fo:mid], xq[:, fo:mid])
                    nc.scalar.dma_start(xq_sb[:, mid:fe], xq[:, mid:fe])
                    for hi in range(1, NHEAD):
                        nc.scalar.dma_start(wq_sb[:, hi], wq[hi])
                    nc.scalar.dma_start(bq_sb[:], bq[:])
                    nc.scalar.dma_start(wo_sb[:], wo[:])
                else:
                    nc.sync.dma_start(xk_sb[:, fo:fe], xk[:, fo:fe])
                    nc.sync.dma_start(xq_sb[:, fo:fe], xq[:, fo:fe])
                    nc.sync.dma_start(xv_sb[:, fo:fe], xv[:, fo:fe])

            def xsl(x_sb, ci, dt_i, lo, hi_):
                """AP for x columns [lo,hi_) of chunk ci's dt tile."""
                _, qw = CHUNKS[ci]
                base = _XOFF[ci] + dt_i * qw
                return x_sb[:, base + lo:base + hi_]

            # ---- PE clock pre-warm: dummy matmuls on memset data ----
            nc.gpsimd.memset(warm_sb[:], 0.0)
            for wi in range(NWARM):
                wp = ps_a.tile([128, 512], F32, tag="ps_a", name=f"warm{wi}")
                nc.tensor.matmul(
                    wp, warm_sb[:, 0:128], warm_sb[:, 128:128 + 512],
                    start=True, stop=True,
                )

            for ci, (q0, qw) in enumerate(CHUNKS):
                sl = slice(q0, q0 + qw)
                nj = qw // 128           # q sub-tiles in this chunk
                jt0 = q0 // 128          # first q tile index
                nkv_c = jt0 + nj         # kv tiles visible to this chunk

                load_chunk(ci)

                # ---- K projection ----
                pk = ps_a.tile([128, qw], F32, tag="ps_a", name=f"pk{ci}")
                for dt_i in range(NDT):
                    nc.tensor.matmul(
                        pk, wk_sb[:, dt_i, :], xsl(xk_sb, ci, dt_i, 0, qw),
                        start=(dt_i == 0), stop=(dt_i == NDT - 1),
                    )
                nc.vector.tensor_tensor(
                    k_sb[:, sl], pk, bk_sb[:].to_broadcast((128, qw)),
                    mybir.AluOpType.add,
                )

                # ---- Q projection (per head) ----
                for hi in range(NHEAD):
                    pq = ps_a.tile([128, qw], F32, tag="ps_a",
                                   name=f"pq{ci}_{hi}")
                    for dt_i in range(NDT):
                        nc.tensor.matmul(
                            pq, wq_sb[:, hi, dt_i, :],
                            xsl(xq_sb, ci, dt_i, 0, qw),
                            start=(dt_i == 0), stop=(dt_i == NDT - 1),
                        )
                    nc.vector.tensor_tensor(
                        q_sb[:, hi, sl], pq,
                        bq_sb[:, hi:hi + 1].to_broadcast((128, qw)),
                        mybir.AluOpType.add,
                    )

                # ---- V projection (per kv tile of this chunk) ----
                for kvs in range(nj):
                    kv = jt0 + kvs
                    pv = ps_a.tile([128, DV + 1], F32, tag="ps_a",
                                   name=f"pv{ci}_{kvs}")
                    for dt_i in range(NDT):
                        nc.tensor.matmul(
                            pv, xsl(xv_sb, ci, dt_i, kvs * 128,
                                    (kvs + 1) * 128),
                            wv_sb[:, dt_i, :],
                            start=(dt_i == 0), stop=(dt_i == NDT - 1),
                        )
                    nc.vector.tensor_tensor(
                        v_sb[:, kv, :], pv, bvb_sb[:], mybir.AluOpType.add
                    )

                # ---- attention for this chunk (kv tiles 0..nkv_c-1) ----
                ctxT = ctpool.tile([128, NHEAD, nj, 128], F16, tag="ctxT",
                                   name=f"ctxT{ci}")
                for hi in range(NHEAD):
                    ctx_ps = [
                        ps_ctx.tile([128, DV + 1], F32, tag="ctx",
                                    name=f"ctx_{ci}_{hi}_{j}")
                        for j in range(nj)
                    ]
                    for kv in range(nkv_c):
                        t_off = kv * 128 - q0   # kv tile col offset in chunk
                        # causal: q columns below kv tile start are all
                        # masked -> shrink score/exp width to the live part
                        qoff = max(t_off, 0)
                        w = qw - qoff
                        s_ps = ps_a.tile([128, qw], F32, tag="ps_a",
                                         name=f"s_{ci}_{hi}_{kv}")
                        nc.tensor.matmul(
                            s_ps[:, :w],
                            k_sb[:, kv * 128:(kv + 1) * 128],
                            q_sb[:, hi, q0 + qoff:q0 + qw],
                            start=True, stop=True,
                        )
                        e_sb = wpool.tile([128, qw], F16, tag="e",
                                          name=f"e_{ci}_{hi}_{kv}")
                        nc.scalar.activation(
                            e_sb[:, :w], s_ps[:, :w],
                            mybir.ActivationFunctionType.Exp,
                            bias=0.0, scale=scale_val,
                        )
                        if t_off >= 0:
                            # only the leading 128 block straddles the
                            # diagonal; later blocks are fully allowed
                            em_sb = mpool.tile([128, 128], F16, tag="em")
                            nc.vector.tensor_tensor(
                                em_sb[:], e_sb[:, 0:128], msk_sb[:],
                                mybir.AluOpType.mult,
                            )
                        for j in range(nj):
                            if kv > jt0 + j:
                                continue
                            if t_off == j * 128:
                                e_use = em_sb[:, 0:128]
                            else:
                                e_use = e_sb[:, j * 128 - qoff:
                                             (j + 1) * 128 - qoff]
                            nc.tensor.matmul(
                                ctx_ps[j],
                                e_use,
                                v_sb[:, kv, :],
                                start=(kv == 0), stop=(kv == jt0 + j),
                            )
                    for j in range(nj):
                        rcp = wpool.tile([128, 1], F32, tag="rcp")
                        nc.vector.reciprocal(rcp[:], ctx_ps[j][:, DV:DV + 1])
                        ctxn = wpool.tile([128, 128], F16, tag="ctxn")
                        nc.vector.tensor_tensor(
                            ctxn[:], ctx_ps[j][:, 0:DV],
                            rcp[:].to_broadcast((128, DV)),
                            mybir.AluOpType.mult,
                        )
                        tr_ps = ps_tr.tile([128, 128], F16, tag="tr")
                        nc.tensor.transpose(tr_ps, ctxn[:], idn_sb[:])
                        nc.vector.tensor_copy(ctxT[:, hi, j, :], tr_ps)

                # ---- out projection for this q chunk ----
                for j in range(nj):
                    o_sb = opool.tile([128, D], F16, tag="o")
                    for nch in range(2):
                        po = ps_a.tile([128, 512], F32, tag="ps_a",
                                       name=f"po{ci}_{j}_{nch}")
                        for hi in range(NHEAD):
                            nc.tensor.matmul(
                                po,
                                ctxT[:, hi, j, :],
                                wo_sb[:, hi, nch * 512:(nch + 1) * 512],
                                start=(hi == 0), stop=(hi == NHEAD - 1),
                            )
                        nc.vector.tensor_copy(
                            o_sb[:, nch * 512:(nch + 1) * 512], po
                        )
                        qt = jt0 + j
                        nc.sync.dma_start(
                            out[qt * 128:(qt + 1) * 128,
                                nch * 512:(nch + 1) * 512],
                            o_sb[:, nch * 512:(nch + 1) * 512],
                        )

    nc.finalize()
    return nc


_NC_CACHE: dict[float, bass.Bass] = {}


def _get_nc(scale_val: float) -> bass.Bass:
    if scale_val not in _NC_CACHE:
        _NC_CACHE[scale_val] = _build(scale_val)
    return _NC_CACHE[scale_val]


def _part_tile(a: np.ndarray) -> np.ndarray:
    """[K, F] -> [128, K//128, F] partition-tiled fp16 contiguous."""
    k, f = a.shape
    return np.ascontiguousarray(
        a.reshape(k // 128, 128, f).transpose(1, 0, 2).astype(np.float16)
    )


def _chunk_flat(a: np.ndarray) -> np.ndarray:
    """[D, L] -> [128, sum(NDT*qw)] chunk-major flat partition-tiled fp16."""
    pt = _part_tile(a)  # [128, NDT, L]
    parts = [
        pt[:, :, q0:q0 + qw].reshape(128, NDT * qw) for q0, qw in CHUNKS
    ]
    return np.ascontiguousarray(np.concatenate(parts, axis=1))


def run(inputs: dict, trace: bool = False):
    in_q = np.asarray(inputs["in_q"], np.float32)
    in_k = np.asarray(inputs["in_k"], np.float32)
    in_v = np.asarray(inputs["in_v"], np.float32)
    Wq = np.asarray(inputs["Wq"], np.float32)
    Wk = np.asarray(inputs["Wk"], np.float32)
    Wv = np.asarray(inputs["Wv"], np.float32)
    Wo = np.asarray(inputs["Wo"], np.float32)
    bq = np.asarray(inputs["bq"], np.float32)
    bk = np.asarray(inputs["bk"], np.float32)
    bv = np.asarray(inputs["bv"], np.float32)
    bo = np.asarray(inputs["bo"], np.float32)
    qes = float(np.asarray(inputs["q_extra_scale"], np.float32).reshape(-1)[0])

    scale_val = qes / float(np.sqrt(DQK))
    nc = _get_nc(scale_val)

    # triangular mask for the single diagonal 128x128 block
    ii = np.arange(128)[:, None]
    jj = np.arange(128)[None, :]
    masks = (jj >= ii).astype(np.float16)  # [128, 128], 1 where q >= kv
    idn = np.eye(128, dtype=np.float16)

    in_maps = []
    for c in range(NCORES):
        b, g, hh = c // 4, (c % 4) // 2, c % 2
        h0 = g * HPG + hh * NHEAD
        wv_aug = np.concatenate(
            [Wv[:, g * DV:(g + 1) * DV], np.zeros((D, 1), np.float32)], axis=1
        )
        bv_aug = np.concatenate([bv[g * DV:(g + 1) * DV], [1.0]]).astype(np.float32)
        wo_slice = Wo[h0 * DV:(h0 + NHEAD) * DV, :]  # [512, 1024]
        in_maps.append({
            "xqT": _chunk_flat(in_q[b].T),
            "xkT": _chunk_flat(in_k[b].T),
            "xvT": _chunk_flat(in_v[b].T),
            "wq": np.stack([
                _part_tile(Wq[:, (h0 + h) * DQK:(h0 + h + 1) * DQK])
                for h in range(NHEAD)
            ]),
            "wk": _part_tile(Wk[:, g * DQK:(g + 1) * DQK]),
            "wv": _part_tile(wv_aug),
            "wo": np.ascontiguousarray(
                wo_slice.reshape(NHEAD, DV, D).transpose(1, 0, 2).astype(np.float16)
            ),
            "bq": np.ascontiguousarray(
                bq[h0 * DQK:(h0 + NHEAD) * DQK].reshape(NHEAD, DQK).T.astype(np.float32)
            ),
            "bk": bk[g * DQK:(g + 1) * DQK].reshape(DQK, 1).astype(np.float32),
            "bvb": np.ascontiguousarray(
                np.broadcast_to(bv_aug, (128, DV + 1)).astype(np.float32)
            ),
            "msk": masks,
            "idn": idn,
        })

    res = run_bass_kernel_spmd(
        nc, in_maps, core_ids=list(range(NCORES)), trace=trace
    )

    out_full = np.zeros((B, L, D), np.float32)
    for c in range(NCORES):
        out_full[c // 4] += np.asarray(res.results[c]["out"], np.float32)
    out_full += bo
    return out_full, res.exec_time_ns


def kernel(**inputs) -> np.ndarray:
    out, _ = run(inputs, trace=False)
    return out


# revision 24
# speedup vs baseline: 1.2458x; 1.0030x over previous
"""Trainium2 Bass kernel for GQA causal attention (nn_Attention_83623013253180).

Shapes: B=2, L=2048, D=1024, H=16 heads, G=2 kv-groups, HPG=8, DQK=DV=128.

Sharding (8 cores): core c -> (b = c//4, g = (c%4)//2, hh = c%2), each core
handles one batch, one kv group, and 4 of that group's 8 query heads.
Wq/Wk/Wv are column-sharded, Wo row-sharded; the out-proj all-reduce (sum of
4 partials per batch) is done on host after gather, along with + bo.

Per-core device kernel (all matmul operands fp16, PSUM fp32), organized as a
pipeline over q chunks (small first chunks so attention starts as soon as a
sliver of x has landed):
  chunk: load x columns -> kT/qT/V_aug projections -> causal attention for
  the chunk (kv tiles 0..end) -> out projection -> store fp16.

  - x arrives chunk-major from host: [128, sum(8*qw)] fp16, contiguous per
    chunk so every load is a single wide DMA row per partition
  - qT[h] = (Wq_h^T X^T)  [128 dqk, tok]   (lhsT=Wq tile, rhs=xT)
  - S^T tile = matmul(lhsT=kT slice, rhs=qT cols) -> PSUM [128 kv, qw]
  - e = exp(S^T * scale) on ScalarE -> fp16 SBUF; causal 0/1 mask multiply
    on diagonal tiles (DVE)
  - ctx PSUM [q 128, 129] += matmul(lhsT=e slice, rhs=V_aug tile); V_aug has
    a ones column so col 128 accumulates the softmax denominator
  - normalize per-partition via reciprocal, PE-transpose ctx -> ctxT
  - out[q,1024] partial = sum_h matmul(lhsT=ctxT_h, rhs=Wo_h) -> DMA fp16

Warmup: dummy matmuls on memset data ramp the PE clock while the first DMAs
stream; chunk-0 loads are split across both HWDGE trigger queues
(sync+scalar) and halved so projections start on partial data.
"""

import numpy as np

import concourse.bass as bass
import concourse.mybir as mybir
import concourse.tile as tile
from concourse import bacc
from concourse.bass_utils import run_bass_kernel_spmd

F16 = mybir.dt.float16
F32 = mybir.dt.float32

B, L, D = 2, 2048, 1024
H, G, HPG = 16, 2, 8
DQK = DV = 128
NHEAD = 4          # heads per core
NDT = D // 128     # 8 contraction tiles over input dim
NKV = L // 128     # 16 kv tiles
NCORES = 8
NWARM = 5          # dummy matmuls to ramp the PE clock during warmup
CHUNKS = [(0, 256), (256, 256), (512, 512), (1024, 512), (1536, 512)]
# flat chunk-major column offset of each chunk in the x dram tensors
_XOFF = [0]
for _, _qw in CHUNKS:
    _XOFF.append(_XOFF[-1] + NDT * _qw)
XTOT = _XOFF[-1]   # == NDT * L


def _build(scale_val: float) -> bass.Bass:
    nc = bacc.Bacc("TRN2", target_bir_lowering=False, debug=False, num_devices=NCORES)

    xq = nc.dram_tensor("xqT", [128, XTOT], F16, kind="ExternalInput")
    xk = nc.dram_tensor("xkT", [128, XTOT], F16, kind="ExternalInput")
    xv = nc.dram_tensor("xvT", [128, XTOT], F16, kind="ExternalInput")
    wq = nc.dram_tensor("wq", [NHEAD, 128, NDT, DQK], F16, kind="ExternalInput")
    wk = nc.dram_tensor("wk", [128, NDT, DQK], F16, kind="ExternalInput")
    wv = nc.dram_tensor("wv", [128, NDT, DV + 1], F16, kind="ExternalInput")
    wo = nc.dram_tensor("wo", [128, NHEAD, D], F16, kind="ExternalInput")
    bq = nc.dram_tensor("bq", [128, NHEAD], F32, kind="ExternalInput")
    bk = nc.dram_tensor("bk", [128, 1], F32, kind="ExternalInput")
    bvb = nc.dram_tensor("bvb", [128, DV + 1], F32, kind="ExternalInput")
    msk = nc.dram_tensor("msk", [128, 128], F16, kind="ExternalInput")
    idn = nc.dram_tensor("idn", [128, 128], F16, kind="ExternalInput")
    out = nc.dram_tensor("out", [L, D], F16, kind="ExternalOutput")

    with tile.TileContext(nc) as tc:
        with (
            tc.tile_pool(name="const", bufs=1) as cpool,
            tc.tile_pool(name="xbuf", bufs=1) as xpool,
            tc.tile_pool(name="qkv", bufs=1) as qkvpool,
            tc.tile_pool(name="work", bufs=8) as wpool,
            tc.tile_pool(name="masked", bufs=4) as mpool,
            tc.tile_pool(name="ctxt", bufs=3) as ctpool,
            tc.tile_pool(name="outb", bufs=4) as opool,
            tc.tile_pool(name="ps_a", bufs=3, space="PSUM") as ps_a,
            tc.tile_pool(name="ps_ctx", bufs=4, space="PSUM") as ps_ctx,
            tc.tile_pool(name="ps_tr", bufs=1, space="PSUM") as ps_tr,
        ):
            wk_sb = cpool.tile([128, NDT, DQK], F16, tag="wk")
            bk_sb = cpool.tile([128, 1], F32, tag="bk")
            bq_sb = cpool.tile([128, NHEAD], F32, tag="bq")
            bvb_sb = cpool.tile([128, DV + 1], F32, tag="bvb")
            msk_sb = cpool.tile([128, 128], F16, tag="msk")
            idn_sb = cpool.tile([128, 128], F16, tag="idn")
            wq_sb = cpool.tile([128, NHEAD, NDT, DQK], F16, tag="wq")
            wv_sb = cpool.tile([128, NDT, DV + 1], F16, tag="wv")
            wo_sb = cpool.tile([128, NHEAD, D], F16, tag="wo")
            warm_sb = cpool.tile([128, 128 + 512], F16, tag="warm")

            q_sb = qkvpool.tile([128, NHEAD, L], F16, tag="q")    # qT per head
            k_sb = qkvpool.tile([128, L], F16, tag="k")           # kT
            v_sb = qkvpool.tile([128, NKV, DV + 1], F16, tag="v")  # V_aug tiles

            xq_sb = xpool.tile([128, XTOT], F16, tag="xq")
            xk_sb = xpool.tile([128, XTOT], F16, tag="xk")
            xv_sb = xpool.tile([128, XTOT], F16, tag="xv")

            def load_chunk(ci):
                fo, fe = _XOFF[ci], _XOFF[ci + 1]
                if ci == 0:
                    # chunk 0 is latency-critical: triggers split across the
                    # two HWDGE queues (sync + scalar), x split in half so
                    # compute starts on partial data
                    mid = (fo + fe) // 2
                    nc.sync.dma_start(wk_sb[:], wk[:])
                    nc.sync.dma_start(xk_sb[:, fo:mid], xk[:, fo:mid])
                    nc.sync.dma_start(xk_sb[:, mid:fe], xk[:, mid:fe])
                    nc.sync.dma_start(wv_sb[:], wv[:])
                    nc.sync.dma_start(xv_sb[:, fo:mid], xv[:, fo:mid])
                    nc.sync.dma_start(xv_sb[:, mid:fe], xv[:, mid:fe])
                    nc.sync.dma_start(bk_sb[:], bk[:])
                    nc.sync.dma_start(bvb_sb[:], bvb[:])
                    nc.sync.dma_start(msk_sb[:], msk[:])
                    nc.sync.dma_start(idn_sb[:], idn[:])
                    nc.scalar.dma_start(wq_sb[:, 0], wq[0])
                    nc.scalar.dma_start(xq_sb[:, fo:mid], xq[:, fo:mid])
                    nc.scalar.dma_start(xq_sb[:, mid:fe], xq[:, mid:fe])
                    for hi in range(1, NHEAD):
                        nc.scalar.dma_start(wq_sb[:, hi], wq[hi])
                    nc.scalar.dma_start(bq_sb[:], bq[:])
                    nc.scalar.dma_start(wo_sb[:], wo[:])
                else:
                    nc.sync.dma_start(xk_sb[:, fo:fe], xk[:, fo:fe])
                    nc.sync.dma_start(xq_sb[:, fo:fe], xq[:, fo:fe])
                    nc.sync.dma_start(xv_sb[:, fo:fe], xv[:, fo:fe])

            def xsl(x_sb, ci, dt_i, lo, hi_):
                """AP for x columns [lo,hi_) of chunk ci's dt tile."""
                _, qw = CHUNKS[ci]
                base = _XOFF[ci] + dt_i * qw
                return x_sb[:, base + lo:base + hi_]

            # ---- PE clock pre-warm: dummy matmuls on memset data ----
            nc.gpsimd.memset(warm_sb[:], 0.0)
            for wi in range(NWARM):
                wp = ps_a.tile([128, 512], F32, tag="ps_a", name=f"warm{wi}")
                nc.tensor.matmul(
                    wp, warm_sb[:, 0:128], warm_sb[:, 128:128 + 512],
                    start=True, stop=True,
                )

            # Projections and out-projections are emitted as ATOMIC units
            # (one psum chain + its evacuation) drawn from a queue and
            # dripped into the attention stream, so ScalarE keeps streaming
            # exps while the PE chews PE-only work in its exp-wait bubbles.
            unit_q = []

            def flush(n):
                for _ in range(min(n, len(unit_q))):
                    unit_q.pop(0)()

            def make_proj_units(ci):
                q0, qw = CHUNKS[ci]
                sl = slice(q0, q0 + qw)
                nj = qw // 128
                jt0 = q0 // 128
                units = []

                def ku():
                    pk = ps_a.tile([128, qw], F32, tag="ps_a",
                                   name=f"pk{ci}")
                    for dt_i in range(NDT):
                        nc.tensor.matmul(
                            pk, wk_sb[:, dt_i, :],
                            xsl(xk_sb, ci, dt_i, 0, qw),
                            start=(dt_i == 0), stop=(dt_i == NDT - 1),
                        )
                    nc.vector.tensor_tensor(
                        k_sb[:, sl], pk, bk_sb[:].to_broadcast((128, qw)),
                        mybir.AluOpType.add,
                    )
                units.append(ku)

                def qu(hi):
                    pq = ps_a.tile([128, qw], F32, tag="ps_a",
                                   name=f"pq{ci}_{hi}")
                    for dt_i in range(NDT):
                        nc.tensor.matmul(
                            pq, wq_sb[:, hi, dt_i, :],
                            xsl(xq_sb, ci, dt_i, 0, qw),
                            start=(dt_i == 0), stop=(dt_i == NDT - 1),
                        )
                    nc.vector.tensor_tensor(
                        q_sb[:, hi, sl], pq,
                        bq_sb[:, hi:hi + 1].to_broadcast((128, qw)),
                        mybir.AluOpType.add,
                    )
                for hi in range(NHEAD):
                    units.append(lambda hi=hi: qu(hi))

                def vu(kvs):
                    kv = jt0 + kvs
                    pv = ps_a.tile([128, DV + 1], F32, tag="ps_a",
                                   name=f"pv{ci}_{kvs}")
                    for dt_i in range(NDT):
                        nc.tensor.matmul(
                            pv, xsl(xv_sb, ci, dt_i, kvs * 128,
                                    (kvs + 1) * 128),
                            wv_sb[:, dt_i, :],
                            start=(dt_i == 0), stop=(dt_i == NDT - 1),
                        )
                    nc.vector.tensor_tensor(
                        v_sb[:, kv, :], pv, bvb_sb[:], mybir.AluOpType.add
                    )
                for kvs in range(nj):
                    units.append(lambda kvs=kvs: vu(kvs))
                return units

            def make_out_units(ci, ctxT, jt0, nj):
                def ou(j, nch):
                    po = ps_a.tile([128, 512], F32, tag="ps_a",
                                   name=f"po{ci}_{j}_{nch}")
                    for hi in range(NHEAD):
                        nc.tensor.matmul(
                            po,
                            ctxT[:, hi, j, :],
                            wo_sb[:, hi, nch * 512:(nch + 1) * 512],
                            start=(hi == 0), stop=(hi == NHEAD - 1),
                        )
                    o_sb = opool.tile([128, 512], F16, tag="o")
                    nc.vector.tensor_copy(o_sb[:], po)
                    qt = jt0 + j
                    nc.sync.dma_start(
                        out[qt * 128:(qt + 1) * 128,
                            nch * 512:(nch + 1) * 512],
                        o_sb[:],
                    )
                return [lambda j=j, nch=nch: ou(j, nch)
                        for j in range(nj) for nch in range(2)]

            load_chunk(0)
            flush_all = 1 << 30
            for u in make_proj_units(0):
                u()

            for ci, (q0, qw) in enumerate(CHUNKS):
                nj = qw // 128           # q sub-tiles in this chunk
                jt0 = q0 // 128          # first q tile index
                nkv_c = jt0 + nj         # kv tiles visible to this chunk

                if ci + 1 < len(CHUNKS):
                    load_chunk(ci + 1)

                # ---- attention for this chunk (kv tiles 0..nkv_c-1),
                # with queued proj/out-proj units dripped in every 4th kv
                # tile ----
                ctxT = ctpool.tile([128, NHEAD, nj, 128], F16, tag="ctxT",
                                   name=f"ctxT{ci}")
                for hi in range(NHEAD):
                    if hi == 2 and ci + 1 < len(CHUNKS):
                        # enqueue next chunk's projections once its x DMAs
                        # have had half the window to land
                        unit_q.extend(make_proj_units(ci + 1))
                    ctx_ps = [
                        ps_ctx.tile([128, DV + 1], F32, tag="ctx",
                                    name=f"ctx_{ci}_{hi}_{j}")
                        for j in range(nj)
                    ]
                    for kv in range(nkv_c):
                        t_off = kv * 128 - q0   # kv tile col offset in chunk
                        # causal: q columns below kv tile start are all
                        # masked -> shrink score/exp width to the live part
                        qoff = max(t_off, 0)
                        w = qw - qoff
                        s_ps = ps_a.tile([128, qw], F32, tag="ps_a",
                                         name=f"s_{ci}_{hi}_{kv}")
                        nc.tensor.matmul(
                            s_ps[:, :w],
                            k_sb[:, kv * 128:(kv + 1) * 128],
                            q_sb[:, hi, q0 + qoff:q0 + qw],
                            start=True, stop=True,
                        )
                        e_sb = wpool.tile([128, qw], F16, tag="e",
                                          name=f"e_{ci}_{hi}_{kv}")
                        nc.scalar.activation(
                            e_sb[:, :w], s_ps[:, :w],
                            mybir.ActivationFunctionType.Exp,
                            bias=0.0, scale=scale_val,
                        )
                        if t_off >= 0:
                            # only the leading 128 block straddles the
                            # diagonal; later blocks are fully allowed
                            em_sb = mpool.tile([128, 128], F16, tag="em")
                            nc.vector.tensor_tensor(
                                em_sb[:], e_sb[:, 0:128], msk_sb[:],
                                mybir.AluOpType.mult,
                            )
                        for j in range(nj):
                            if kv > jt0 + j:
                                continue
                            if t_off == j * 128:
                                e_use = em_sb[:, 0:128]
                            else:
                                e_use = e_sb[:, j * 128 - qoff:
                                             (j + 1) * 128 - qoff]
                            nc.tensor.matmul(
                                ctx_ps[j],
                                e_use,
                                v_sb[:, kv, :],
                                start=(kv == 0), stop=(kv == jt0 + j),
                            )
                        if kv % 4 == 2:
                            flush(1)
                    for j in range(nj):
                        rcp = wpool.tile([128, 1], F32, tag="rcp")
                        nc.vector.reciprocal(rcp[:], ctx_ps[j][:, DV:DV + 1])
                        ctxn = wpool.tile([128, 128], F16, tag="ctxn")
                        nc.vector.tensor_tensor(
                            ctxn[:], ctx_ps[j][:, 0:DV],
                            rcp[:].to_broadcast((128, DV)),
                            mybir.AluOpType.mult,
                        )
                        tr_ps = ps_tr.tile([128, 128], F16, tag="tr")
                        nc.tensor.transpose(tr_ps, ctxn[:], idn_sb[:])
                        nc.vector.tensor_copy(ctxT[:, hi, j, :], tr_ps)

                # drain leftovers (next chunk's attention needs q/k/v ready)
                flush(flush_all)
                # this chunk's out-projection runs inside the next chunk's
                # attention window
                unit_q.extend(make_out_units(ci, ctxT, jt0, nj))

            flush(flush_all)   # final chunk's out-projection (tail)

    nc.finalize()
    return nc


_NC_CACHE: dict[float, bass.Bass] = {}


def _get_nc(scale_val: float) -> bass.Bass:
    if scale_val not in _NC_CACHE:
        _NC_CACHE[scale_val] = _build(scale_val)
    return _NC_CACHE[scale_val]


def _part_tile(a: np.ndarray) -> np.ndarray:
    """[K, F] -> [128, K//128, F] partition-tiled fp16 contiguous."""
    k, f = a.shape
    return np.ascontiguousarray(
        a.reshape(k // 128, 128, f).transpose(1, 0, 2).astype(np.float16)
    )


def _chunk_flat(a: np.ndarray) -> np.ndarray:
    """[D, L] -> [128, sum(NDT*qw)] chunk-major flat partition-tiled fp16."""
    pt = _part_tile(a)  # [128, NDT, L]
    parts = [
        pt[:, :, q0:q0 + qw].reshape(128, NDT * qw) for q0, qw in CHUNKS
    ]
    return np.ascontiguousarray(np.concatenate(parts, axis=1))


def run(inputs: dict, trace: bool = False):
    in_q = np.asarray(inputs["in_q"], np.float32)
    in_k = np.asarray(inputs["in_k"], np.float32)
    in_v = np.asarray(inputs["in_v"], np.float32)
    Wq = np.asarray(inputs["Wq"], np.float32)
    Wk = np.asarray(inputs["Wk"], np.float32)
    Wv = np.asarray(inputs["Wv"], np.float32)
    Wo = np.asarray(inputs["Wo"], np.float32)
    bq = np.asarray(inputs["bq"], np.float32)
    bk = np.asarray(inputs["bk"], np.float32)
    bv = np.asarray(inputs["bv"], np.float32)
    bo = np.asarray(inputs["bo"], np.float32)
    qes = float(np.asarray(inputs["q_extra_scale"], np.float32).reshape(-1)[0])

    scale_val = qes / float(np.sqrt(DQK))
    nc = _get_nc(scale_val)

    # triangular mask for the single diagonal 128x128 block
    ii = np.arange(128)[:, None]
    jj = np.arange(128)[None, :]
    masks = (jj >= ii).astype(np.float16)  # [128, 128], 1 where q >= kv
    idn = np.eye(128, dtype=np.float16)

    in_maps = []
    for c in range(NCORES):
        b, g, hh = c // 4, (c % 4) // 2, c % 2
        h0 = g * HPG + hh * NHEAD
        wv_aug = np.concatenate(
            [Wv[:, g * DV:(g + 1) * DV], np.zeros((D, 1), np.float32)], axis=1
        )
        bv_aug = np.concatenate([bv[g * DV:(g + 1) * DV], [1.0]]).astype(np.float32)
        wo_slice = Wo[h0 * DV:(h0 + NHEAD) * DV, :]  # [512, 1024]
        in_maps.append({
            "xqT": _chunk_flat(in_q[b].T),
            "xkT": _chunk_flat(in_k[b].T),
            "xvT": _chunk_flat(in_v[b].T),
            "wq": np.stack([
                _part_tile(Wq[:, (h0 + h) * DQK:(h0 + h + 1) * DQK])
                for h in range(NHEAD)
            ]),
            "wk": _part_tile(Wk[:, g * DQK:(g + 1) * DQK]),
            "wv": _part_tile(wv_aug),
            "wo": np.ascontiguousarray(
                wo_slice.reshape(NHEAD, DV, D).transpose(1, 0, 2).astype(np.float16)
            ),
            "bq": np.ascontiguousarray(
                bq[h0 * DQK:(h0 + NHEAD) * DQK].reshape(NHEAD, DQK).T.astype(np.float32)
            ),
            "bk": bk[g * DQK:(g + 1) * DQK].reshape(DQK, 1).astype(np.float32),
            "bvb": np.ascontiguousarray(
                np.broadcast_to(bv_aug, (128, DV + 1)).astype(np.float32)
            ),
            "msk": masks,
            "idn": idn,
        })

    res = run_bass_kernel_spmd(
        nc, in_maps, core_ids=list(range(NCORES)), trace=trace
    )

    out_full = np.zeros((B, L, D), np.float32)
    for c in range(NCORES):
        out_full[c // 4] += np.asarray(res.results[c]["out"], np.float32)
    out_full += bo
    return out_full, res.exec_time_ns


def kernel(**inputs) -> np.ndarray:
    out, _ = run(inputs, trace=False)
    return out
